# revision 9
# baseline (speedup 1.0000x reference)
"""Multi-head causal self-attention on 8 Trainium2 NeuronCores.

Full-tensor contract: kernel(**inputs) takes the unsharded inputs of
reference.setup_inputs() and returns the full [B, S, D] output.

Sharding: core c handles (batch b = c // 2, head-group hg = c % 2).
Each head-group = 8 heads = 512 columns of Wq/Wk/Wv (and 512 rows of Wm).
Each core computes a partial output projection y_partial[b] and the host
sums the two head-group partials per batch and adds bm.

Per-core dataflow (everything oriented so matmuls contract over the
partition dim with no on-device transposes):
  xT     [D, S] = x[b].T (host-transposed), streamed as column panels
         [128, 8x512] so each projection PSUM accumulation completes
         within one panel
  QT,KT  [128, S] per head-pair (2 heads x 64 rows) = (W_pair)^T x^T,
         Q pre-scaled by 1/8, biases added per-partition on ScalarE
  V      [128, k-tiles x (8*65)] natural [k, head-cols] with a ones
         column per head (gives the softmax denominator for free)
  S^T    [k-tile 128, q-chunk 512] = K (Q/8)^T (scores, transposed);
         fully-masked k-tiles skipped, diagonal tiles get a causal
         band added from a precomputed [128, 896] tile
  P^T    = exp(S^T) on ScalarE (no max-subtraction: |scores| <= ~60 so
         fp32 exp cannot overflow; masked entries exp(-1e12) = 0)
  U      [65, 512] = [V | 1]^T P^T accumulated over k-tiles in PSUM;
         row 64 is the softmax denominator
  attnT  [hc, q] = U[0:64] * bcast(1/U[64]) (rank-1 PE matmul
         broadcasts the reciprocal across partitions)
  y      [S, D] = attnT-chunks^T @ Wm-chunks (partial; host sums)

All matmul operands are viewed as float32r: full PE rate at N=512
with near-fp32 precision.
"""

import numpy as np

import concourse.bacc as bacc
import concourse.tile as tile
from concourse import mybir
from concourse import bass_utils

B, S, D, H = 4, 2048, 1024, 16
HS = 64
N_CORES = 8
NEG = -1.0e12

F32 = mybir.dt.float32
F32R = mybir.dt.float32r
AF = mybir.ActivationFunctionType


def r(ap):
    """View an AP as float32r for full-rate fp32 matmuls."""
    return ap.bitcast(F32R)


def build_attention_nc(SC, DC, HL, QCH, och=512):
    """Build the per-core Bass program.

    SC: sequence length, DC: model dim, HL: local heads,
    QCH: q-chunk width (moving free dim), och: output column chunk.
    Returns a Bacc instance (caller runs nc.compile()).
    """
    HG = HL * HS            # local head columns
    NPAIR = HL // 2         # head pairs (128 partition rows each)
    DCH = DC // 128         # contraction chunks for projections
    NJ = SC // QCH          # q chunks
    KTILES = SC // 128      # k tiles
    DIAG = QCH // 128       # diagonal (partially masked) k-tiles per q chunk
    OCH = min(och, DC)
    MW = QCH + QCH - 128    # mask band width

    nc = bacc.Bacc("TRN2", target_bir_lowering=False, debug=False)

    xT = nc.dram_tensor("xT", [DC, SC], F32, kind="ExternalInput").ap()
    wq = nc.dram_tensor("wq", [DC, HG], F32, kind="ExternalInput").ap()
    wk = nc.dram_tensor("wk", [DC, HG], F32, kind="ExternalInput").ap()
    wv = nc.dram_tensor("wv", [DC, HG], F32, kind="ExternalInput").ap()
    wm = nc.dram_tensor("wm", [HG, DC], F32, kind="ExternalInput").ap()
    bqp = nc.dram_tensor("bqp", [128, NPAIR], F32, kind="ExternalInput").ap()
    bkp = nc.dram_tensor("bkp", [128, NPAIR], F32, kind="ExternalInput").ap()
    bvb = nc.dram_tensor("bvb", [128, HG], F32, kind="ExternalInput").ap()
    mkb = nc.dram_tensor("mkb", [128, MW], F32, kind="ExternalInput").ap()
    onc = nc.dram_tensor("onc", [128, HS], F32, kind="ExternalInput").ap()
    y = nc.dram_tensor("y", [SC, DC], F32, kind="ExternalOutput").ap()

    VW = HL * (HS + 1)      # per-k-tile V width: heads x 65

    with tile.TileContext(nc) as tc:
        lp = nc.allow_low_precision(reason="float32r matmul operands (fp32-width data)")
        lp.__enter__()
        with (
            tc.tile_pool(name="cst", bufs=1) as cstpool,
            tc.tile_pool(name="vs", bufs=1) as vpool,
            tc.tile_pool(name="qt", bufs=1) as qtpool,
            tc.tile_pool(name="kt", bufs=1) as ktpool,
            tc.tile_pool(name="wm", bufs=1) as wmpool,
            tc.tile_pool(name="ps", bufs=4, space="PSUM") as ps,
        ):
            # --- constants ---
            bq_s = cstpool.tile([128, NPAIR], F32, tag="bq")
            bk_s = cstpool.tile([128, NPAIR], F32, tag="bk")
            bv_s = cstpool.tile([128, HG], F32, tag="bv")
            mk_s = cstpool.tile([128, MW], F32, tag="mk")
            on_s = cstpool.tile([1, HS], F32R, tag="ones")
            nc.sync.dma_start(bq_s[:], bqp[:])
            nc.sync.dma_start(bk_s[:], bkp[:])
            nc.sync.dma_start(bv_s[:], bvb[:])
            nc.sync.dma_start(mk_s[:], mkb[:])
            nc.sync.dma_start(on_s[:], r(onc[0:1, 0:HS]))

            v_s = vpool.tile([128, KTILES * VW], F32R)
            qt_s = qtpool.tile([128, NPAIR * SC], F32R)
            kt_s = ktpool.tile([128, NPAIR * SC], F32R)
            wm_s = wmpool.tile([128, NPAIR * DC], F32R)
            for p in range(NPAIR):
                nc.sync.dma_start(wm_s[:, p * DC:(p + 1) * DC], r(wm[p * 128:(p + 1) * 128, :]))

            # === phase 0: projections (QT, KT, V), xT streamed by q panels ===
            with (
                tc.tile_pool(name="w", bufs=3) as wpool,
                tc.tile_pool(name="xp", bufs=2) as xpool,
            ):
                wq_s = wpool.tile([128, DCH * HG], F32R, tag="w")
                wk_s = wpool.tile([128, DCH * HG], F32R, tag="w")
                wv_s = wpool.tile([128, DCH * HG], F32R, tag="w")
                for d in range(DCH):
                    nc.sync.dma_start(wq_s[:, d * HG:(d + 1) * HG], r(wq[d * 128:(d + 1) * 128, :]))
                    nc.sync.dma_start(wk_s[:, d * HG:(d + 1) * HG], r(wk[d * 128:(d + 1) * 128, :]))
                    nc.sync.dma_start(wv_s[:, d * HG:(d + 1) * HG], r(wv[d * 128:(d + 1) * 128, :]))

                for j in range(NJ):
                    xp = xpool.tile([128, DCH * QCH], F32R, tag="xp")
                    for d in range(DCH):
                        nc.sync.dma_start(xp[:, d * QCH:(d + 1) * QCH],
                                          r(xT[d * 128:(d + 1) * 128, j * QCH:(j + 1) * QCH]))

                    # V for k-tiles covered by this panel
                    for t in range(j * QCH // 128, (j + 1) * QCH // 128):
                        tl = t * 128 - j * QCH  # local column offset in panel
                        vp = ps.tile([128, HG], F32, tag="ps")
                        for d in range(DCH):
                            nc.tensor.matmul(
                                vp[:], r(xp[:, d * QCH + tl: d * QCH + tl + 128]),
                                r(wv_s[:, d * HG:(d + 1) * HG]),
                                start=(d == 0), stop=(d == DCH - 1))
                        vt = v_s[:, t * VW:(t + 1) * VW]
                        vt3 = vt.rearrange("p (h c) -> p h c", c=HS + 1)
                        nc.sync.dma_start(vt3[:, :, HS], r(onc[:, 0:HL]))
                        nc.vector.tensor_add(
                            vt3[:, :, 0:HS],
                            vp.rearrange("p (h c) -> p h c", c=HS),
                            bv_s.rearrange("p (h c) -> p h c", c=HS))

                    # QT / KT for all head pairs at this q chunk
                    for p in range(NPAIR):
                        qp = ps.tile([128, QCH], F32, tag="ps")
                        kp = ps.tile([128, QCH], F32, tag="ps")
                        for d in range(DCH):
                            wqc = wq_s[:, d * HG + p * 128: d * HG + (p + 1) * 128]
                            wkc = wk_s[:, d * HG + p * 128: d * HG + (p + 1) * 128]
                            xc = xp[:, d * QCH:(d + 1) * QCH]
                            nc.tensor.matmul(qp[:], r(wqc), r(xc),
                                             start=(d == 0), stop=(d == DCH - 1))
                            nc.tensor.matmul(kp[:], r(wkc), r(xc),
                                             start=(d == 0), stop=(d == DCH - 1))
                        nc.scalar.activation(qt_s[:, p * SC + j * QCH: p * SC + (j + 1) * QCH],
                                             qp[:], AF.Identity, bias=bq_s[:, p:p + 1], scale=0.125)
                        nc.scalar.activation(kt_s[:, p * SC + j * QCH: p * SC + (j + 1) * QCH],
                                             kp[:], AF.Identity, bias=bk_s[:, p:p + 1], scale=1.0)

            # === attention ===
            with (
                tc.tile_pool(name="at", bufs=1) as atpool,
                tc.tile_pool(name="exp", bufs=3) as epool,
                tc.tile_pool(name="scr", bufs=2) as spool,
                tc.tile_pool(name="u", bufs=4, space="PSUM") as ups,
            ):
                attnT = atpool.tile([128, NPAIR * SC], F32R)

                for p in range(NPAIR):
                    for j in range(NJ):
                        nt = DIAG * (j + 1)  # valid k tiles for this q chunk
                        u0 = ups.tile([HS + 1, QCH], F32, tag="u")
                        u1 = ups.tile([HS + 1, QCH], F32, tag="u")
                        for t in range(nt):
                            for h, uu in enumerate((u0, u1)):
                                plo = p * SC if h == 0 else p * SC  # pair chunk base
                                rlo = 64 * h
                                sp = ps.tile([128, QCH], F32, tag="ps")
                                nc.tensor.matmul(
                                    sp[:],
                                    r(kt_s[rlo:rlo + HS, p * SC + t * 128: p * SC + (t + 1) * 128]),
                                    r(qt_s[rlo:rlo + HS, p * SC + j * QCH: p * SC + (j + 1) * QCH]),
                                    start=True, stop=True)
                                i = t - (nt - DIAG)
                                if i >= 0:  # diagonal tile: add causal band
                                    off = (QCH - 128) - 128 * i
                                    nc.vector.tensor_add(sp[:], sp[:], mk_s[:, off:off + QCH])
                                ex = epool.tile([128, QCH], F32R, tag="exp")
                                nc.scalar.activation(ex[:], sp[:], AF.Exp)
                                vc = v_s[:, t * VW + (2 * p + h) * (HS + 1):
                                         t * VW + (2 * p + h + 1) * (HS + 1)]
                                nc.tensor.matmul(uu[:], r(vc), r(ex[:]),
                                                 start=(t == 0), stop=(t == nt - 1))
                        # normalize: attnT = U[0:64] * bcast(1/U[64])
                        for h, uu in enumerate((u0, u1)):
                            rcp = spool.tile([1, QCH], F32R, tag="rcp")
                            nc.vector.reciprocal(rcp[:], uu[HS:HS + 1, :])
                            bc = ps.tile([HS, QCH], F32, tag="ps")
                            nc.tensor.matmul(bc[:], r(on_s[:]), r(rcp[:]),
                                             start=True, stop=True)
                            uc = spool.tile([HS, QCH], F32, tag="uc")
                            nc.scalar.copy(uc[:], uu[0:HS, :])
                            nc.vector.tensor_mul(
                                attnT[64 * h:64 * h + HS,
                                      p * SC + j * QCH: p * SC + (j + 1) * QCH],
                                uc[:], bc[:])

                # === output projection (partial): y = attnT-chunks^T @ Wm ===
                with tc.tile_pool(name="yst", bufs=3) as ypool:
                    for qt in range(SC // 128):
                        for o in range(DC // OCH):
                            yp = ps.tile([128, OCH], F32, tag="ps")
                            for p in range(NPAIR):
                                nc.tensor.matmul(
                                    yp[:],
                                    r(attnT[:, p * SC + qt * 128: p * SC + (qt + 1) * 128]),
                                    r(wm_s[:, p * DC + o * OCH: p * DC + (o + 1) * OCH]),
                                    start=(p == 0), stop=(p == NPAIR - 1))
                            ys = ypool.tile([128, OCH], F32, tag="yst")
                            nc.scalar.copy(ys[:], yp[:])
                            nc.sync.dma_start(
                                y[qt * 128:(qt + 1) * 128, o * OCH:(o + 1) * OCH], ys[:])

    return nc


def make_mask_band(QCH):
    MW = QCH + QCH - 128
    p = np.arange(128)[:, None]
    jj = np.arange(MW)[None, :]
    return np.where(jj < p + (QCH - 128), np.float32(NEG), np.float32(0.0))


def host_inputs_for_core(x, Wq, bq, Wk, bk, Wv, bv, Wm, core, HL, QCH):
    b, hg = core // 2, core % 2
    HG = HL * HS
    lo, hi = hg * HG, (hg + 1) * HG
    NPAIR = HL // 2
    return {
        "xT": np.ascontiguousarray(x[b].T),
        "wq": np.ascontiguousarray(Wq[:, lo:hi]),
        "wk": np.ascontiguousarray(Wk[:, lo:hi]),
        "wv": np.ascontiguousarray(Wv[:, lo:hi]),
        "wm": np.ascontiguousarray(Wm[lo:hi, :]),
        "bqp": np.ascontiguousarray((bq[lo:hi] / 8.0).reshape(NPAIR, 128).T),
        "bkp": np.ascontiguousarray(bk[lo:hi].reshape(NPAIR, 128).T),
        "bvb": np.broadcast_to(bv[lo:hi], (128, HG)).copy(),
        "mkb": make_mask_band(QCH),
        "onc": np.ones((128, HS), np.float32),
    }


_CACHE = {}


def _get_nc():
    if "nc" not in _CACHE:
        nc = build_attention_nc(S, D, H // 2, 512)
        nc.compile()
        _CACHE["nc"] = nc
    return _CACHE["nc"]


def kernel(x, Wq, bq, Wk, bk, Wv, bv, Wm, bm):
    x = np.asarray(x, np.float32)
    nc = _get_nc()
    in_maps = [
        host_inputs_for_core(x, np.asarray(Wq), np.asarray(bq), np.asarray(Wk),
                             np.asarray(bk), np.asarray(Wv), np.asarray(bv),
                             np.asarray(Wm), c, H // 2, 512)
        for c in range(N_CORES)
    ]
    res = bass_utils.run_bass_kernel_spmd(nc, in_maps, core_ids=list(range(N_CORES)))
    out = np.empty((B, S, D), np.float32)
    bm = np.asarray(bm, np.float32)
    for b in range(B):
        out[b] = res.results[2 * b]["y"] + res.results[2 * b + 1]["y"] + bm
    return out
